# revision 3
# baseline (speedup 1.0000x reference)
"""VQ codebook reconstruction kernel for Trainium2 (8 NeuronCores, SPMD).

Reference computation (per pixel feature vector f in R^C):
    weights = (codebook @ f) / ||codebook_rows||^2      # [N]
    recon   = codebook.T @ weights                      # [C]

This collapses to a single fixed matrix applied per pixel:
    recon = M @ f,   M = codebook.T @ diag(1/||c_n||^2) @ codebook   # [C, C]

M is tiny ([256,256]) and is formed on the host in float64; the device
kernel applies M to all B*H*W = 131072 pixel vectors, sharded
data-parallel over (B, H) across 8 cores. Matmuls use float32r (full
fp32 precision at 1 cycle/row for moving dim >= 256).
"""

import numpy as np

B, C, H, W = 4, 256, 128, 256
N_CORES = 8
SPLIT_H = 2            # 8 shards = B(4) x H-halves(2)
SH = H // SPLIT_H      # 64 rows of H per shard
P_SHARD = SH * W       # 16384 pixels per core
TILE_N = 512
N_TILES = P_SHARD // TILE_N  # 32

_NC_CACHE = {}


def _build_nc():
    if "nc" in _NC_CACHE:
        return _NC_CACHE["nc"]

    import concourse.bass as bass
    import concourse.tile as tile
    from concourse import bacc, mybir

    f32 = mybir.dt.float32
    f32r = mybir.dt.float32r

    nc = bacc.Bacc()
    feat = nc.dram_tensor("feat", [C, P_SHARD], f32r, kind="ExternalInput")
    mmat = nc.dram_tensor("mmat", [C, C], f32r, kind="ExternalInput")
    out = nc.dram_tensor("out", [C, P_SHARD], f32, kind="ExternalOutput")

    with tile.TileContext(nc) as tc:
        with (
            tc.tile_pool(name="mpool", bufs=1) as mpool,
            tc.tile_pool(name="rhs", bufs=4) as rhs_pool,
            tc.tile_pool(name="opool", bufs=4) as opool,
            tc.tile_pool(name="psum", bufs=4, space="PSUM") as psum_pool,
        ):
            # M as two [128, 256] K-halves; lhsT block for (kb, mb) is
            # m_tiles[kb][:, mb*128:(mb+1)*128] (M is symmetric so lhsT = M).
            m_tiles = []
            for kb in range(2):
                mt = mpool.tile([128, C], f32r, tag=f"m{kb}")
                nc.sync.dma_start(mt[:], mmat[kb * 128:(kb + 1) * 128, :])
                m_tiles.append(mt)

            for j in range(N_TILES):
                r = []
                for kb in range(2):
                    rt = rhs_pool.tile([128, TILE_N], f32r, tag=f"r{kb}")
                    nc.sync.dma_start(
                        rt[:], feat[kb * 128:(kb + 1) * 128, bass.ts(j, TILE_N)]
                    )
                    r.append(rt)
                for mb in range(2):
                    ps = psum_pool.tile([128, TILE_N], f32, tag=f"ps{mb}")
                    for kb in range(2):
                        nc.tensor.matmul(
                            ps[:],
                            m_tiles[kb][:, mb * 128:(mb + 1) * 128],
                            r[kb][:],
                            start=(kb == 0),
                            stop=(kb == 1),
                        )
                    ot = opool.tile([128, TILE_N], f32, tag=f"o{mb}")
                    if mb == 0:
                        nc.vector.tensor_copy(ot[:], ps[:])
                    else:
                        nc.scalar.copy(ot[:], ps[:])
                    nc.sync.dma_start(
                        out[mb * 128:(mb + 1) * 128, bass.ts(j, TILE_N)], ot[:]
                    )

    nc.compile()
    _NC_CACHE["nc"] = nc
    return nc


def _host_prep(feature, codebook):
    cb = codebook.astype(np.float64)
    norm = np.sum(cb * cb, axis=1)
    m = ((cb / norm[:, None]).T @ cb).astype(np.float32)

    in_maps = []
    for i in range(N_CORES):
        b, hs = i // SPLIT_H, (i % SPLIT_H) * SH
        shard = np.ascontiguousarray(
            feature[b, :, hs:hs + SH, :].reshape(C, P_SHARD)
        )
        in_maps.append({"feat": shard, "mmat": m})
    return in_maps


def _gather(results):
    out = np.empty((B, C, H, W), dtype=np.float32)
    for i in range(N_CORES):
        b, hs = i // SPLIT_H, (i % SPLIT_H) * SH
        out[b, :, hs:hs + SH, :] = results[i]["out"].reshape(C, SH, W)
    return out


def run(feature, codebook, **spmd_kwargs):
    from concourse.bass_utils import run_bass_kernel_spmd

    nc = _build_nc()
    in_maps = _host_prep(np.asarray(feature), np.asarray(codebook))
    res = run_bass_kernel_spmd(nc, in_maps, list(range(N_CORES)), **spmd_kwargs)
    return _gather(res.results), res


def kernel(feature, codebook):
    out, _ = run(feature, codebook)
    return out
